# revision 37
# baseline (speedup 1.0000x reference)
"""Trainium2 Bass kernel for an nn.AttentionBlock (GroupNorm -> qkv 1x1 conv ->
single-head self-attention over 32x32 spatial tokens -> proj 1x1 conv ->
residual add).

Full-input contract: kernel(**inputs) takes the complete B=16 batch and
returns the full [16, 512, 32, 32] output. Internally the batch is sharded
2-samples-per-core over 8 NeuronCores (pure data parallelism, no
collectives); the small channel-dim weights are replicated.

v4 strategy (fp8 DoubleRow pipeline + deep schedule restructuring):
  All large matmuls run in float8e4 with MatmulPerfMode.DoubleRow.  Host
  foldings: M = Wq^T Wk (scores from one projected tensor), Wpv = Wp Wv
  (proj folded into AV).  When the folded bias vectors are exactly zero
  (they are for this problem: qkv_b = proj_b = 0) the rowbias and
  pb-injection device paths are skipped entirely.
  x is loaded in bf16 (halves HBM time); GroupNorm runs as four
  independent per-channel-chunk chains (groups never span chunks), with
  small ops spread over Scalar/GpSimd so the Vector queue carries only
  bn_stats + evacuations.  GN group matmuls use tiny [128,8]/[8,128]
  indicator operands and share a dedicated PSUM pool so the big-matmul
  pool rotation is never blocked by them.  A short fp8 warm block rides
  the x-load window to keep the PE HAM un-throttled.  Sample-1 AV matmul
  groups are interleaved around the dn/reciprocal chain so the PE never
  idles at the tail; final mult/add/DMA run in 512-column halves across
  Vector/GpSimd/two DMA queues to minimize the post-matmul tail.
"""

import os
import sys
import threading

sys.path.insert(0, "/opt/trn_rl_repo")

import numpy as np
import ml_dtypes

import concourse.bass as bass
import concourse.tile as tile
from concourse import mybir
from concourse.bass_utils import run_bass_kernel_spmd

# ---------------------------------------------------------------------------
# Workaround for this walrus build: CoreV3 codegen accepts at most ONE sync
# wait per instruction.  The Tile scheduler freely attaches several.
# Post-pass: hoist all but the last wait of each instruction onto preceding
# single-wait NOPs on the same engine.
# ---------------------------------------------------------------------------


def _split_multi_waits(nc, maxw=1):
    seq = 0
    for f in nc.m.functions:
        for bb in f.blocks:
            new_list = []
            changed = False
            for ins in bb.instructions:
                si = getattr(ins, "sync_info", None)
                waits = list(si.on_wait) if si and si.on_wait else []
                if len(waits) > maxw:
                    changed = True
                    for w in waits[:-maxw]:
                        seq += 1
                        new_list.append(
                            mybir.InstNoOp(
                                name=f"I-wsplit-{seq}",
                                engine=ins.engine,
                                sync_info=mybir.SyncInfo(on_wait=[w], on_update=[]),
                                text_hint="wait_split",
                            )
                        )
                    ins.sync_info = mybir.SyncInfo(
                        on_wait=waits[-maxw:], on_update=list(si.on_update)
                    )
                new_list.append(ins)
            if changed:
                bb.instructions[:] = new_list


def _install_axon_ntff_shim():
    """The agent image's `antenv` stub lacks `axon_hooks`, so trace=True would
    be silently skipped.  Recreate the module and register the ctypes-based
    NTFF hook from trn_agent_boot (best effort; timing-only)."""
    try:
        from antenv.axon_hooks import get_axon_ntff_profile_hook  # noqa: F401
        return
    except ImportError:
        pass
    try:
        import types

        import antenv
        from trn_agent_boot.trn_boot import _ntff_profile_via_ctypes

        mod = types.ModuleType("antenv.axon_hooks")
        state = {}
        mod.set_axon_ntff_profile_hook = lambda h: state.__setitem__("h", h)
        mod.get_axon_ntff_profile_hook = lambda: state.get("h")
        sys.modules["antenv.axon_hooks"] = mod
        antenv.axon_hooks = mod
        hook = _ntff_profile_via_ctypes("/opt/axon/libaxon_pjrt.so")
        if hook is not None:
            mod.set_axon_ntff_profile_hook(hook)
    except Exception:
        pass


_install_axon_ntff_shim()

# ---------------------------------------------------------------------------
# Problem constants (hardcoded -- the harness provides no spec files).
# ---------------------------------------------------------------------------

B, C, H, W = 16, 512, 32, 32
N = H * W              # 1024 tokens per sample
GROUPS = 32
GSIZE = C // GROUPS    # 16 channels per group
GPC = 8                # groups per 128-channel chunk
EPS = 1e-5
NCORES = 8
SPC = B // NCORES      # samples per core
P = 128                # partitions
CT = C // P            # 4 channel tiles
NT = N // P            # 8 token tiles
NH = N // 512          # 2 free-dim halves of the token axis
SCALE = 1.0 / np.sqrt(C)
ESHIFT = 2.5           # global exp shift; cancels in softmax ratio

F32 = mybir.dt.float32
F32R = mybir.dt.float32r
BF16 = mybir.dt.bfloat16
F8 = mybir.dt.float8e4
NPF8 = ml_dtypes.float8_e4m3
NPBF16 = ml_dtypes.bfloat16
DR = mybir.MatmulPerfMode.DoubleRow


def _build_program(split_waits=True, use_rowbias=False, use_pb=False,
                   triv_gn=True):
    nc = bass.Bass()

    xs = nc.dram_tensor("xs", [SPC, C, N], BF16, kind="ExternalInput")
    wqk = nc.dram_tensor("wqk", [P, CT, C], F8, kind="ExternalInput")
    wpv = nc.dram_tensor("wpv", [P, CT, C], F8, kind="ExternalInput")
    one8d = nc.dram_tensor("one8", [P, 2, 32], F8, kind="ExternalInput")
    one_r = nc.dram_tensor("one_r", [1, P], F32R, kind="ExternalInput")
    gnw = nc.dram_tensor("gnw", [P, CT, 1], F32, kind="ExternalInput")
    gnb = nc.dram_tensor("gnb", [P, CT, 1], F32, kind="ExternalInput")
    i1d = nc.dram_tensor("ind1", [P, CT, GROUPS], F32, kind="ExternalInput")
    i2d = nc.dram_tensor("ind2", [GROUPS, C], F32, kind="ExternalInput")
    if use_rowbias:
        g8d = nc.dram_tensor("g8", [P, CT, 1], F8, kind="ExternalInput")
    if use_pb:
        pb8d = nc.dram_tensor("pb8", [1, C], F8, kind="ExternalInput")
    out = nc.dram_tensor("out", [SPC, C, N], F32, kind="ExternalOutput")

    AF = mybir.ActivationFunctionType
    OP = mybir.AluOpType

    with tile.TileContext(nc) as tc:
        ctx_lp = nc.allow_low_precision(reason="fp8 matmul pipeline")
        ctx_lp.__enter__()
        with (
            tc.tile_pool(name="wpool", bufs=1) as wpool,
            tc.tile_pool(name="xpool", bufs=2) as xpool,
            tc.tile_pool(name="hpool", bufs=2) as hpool,
            tc.tile_pool(name="tpool", bufs=2) as tpool,
            tc.tile_pool(name="vpool", bufs=2) as vpool,
            tc.tile_pool(name="epool", bufs=2) as epool,
            tc.tile_pool(name="rpool", bufs=2) as rpool,
            tc.tile_pool(name="o1pool", bufs=4) as o1pool,
            tc.tile_pool(name="opool", bufs=4) as opool,
            tc.tile_pool(name="aux", bufs=4) as aux,
            tc.tile_pool(name="pmm", bufs=3, space="PSUM") as pmm,
            tc.tile_pool(name="pdn", bufs=1, space="PSUM") as pdn,
        ):
            # ---- x0 first on both non-SP DMA queues (HBM BW is the floor;
            # SP carries the weights concurrently) -------------------------
            def load_x(s, queues):
                # one DMA per 128-channel chunk: fewer issue slots, and the
                # per-chunk completion semaphore gates bn_stats per chunk
                x_t = xpool.tile([P, CT, N], BF16, tag="x", name=f"x_{s}")
                for ci in range(CT):
                    q = queues[ci % len(queues)]
                    q.dma_start(
                        x_t[:, ci, :],
                        xs[s, ci * P:(ci + 1) * P, :],
                    )
                return x_t

            # x0's first two chunks ride separate queues (concurrent DMAs
            # round-robin HBM packets, so two streams beat one queue's serial
            # ~1.9us/chunk); the GN indicator weights go on SP before chunk 3
            xt0 = load_x(0, [nc.sync, nc.scalar, nc.gpsimd, nc.sync])

            # ---- PE warm block: fp8 DR matmuls on memset junk (no DMA dep)
            # keep the HAM un-throttled through the x-load window.
            wjunk = wpool.tile([P, 2, 512], F8, name="wjunk")
            nc.vector.memset(wjunk[:], 1.0)
            warm_ps = pdn.tile([32, 512], F32, tag="dn", name="warm_ps")
            NWARM = 20
            for wi in range(NWARM):
                nc.tensor.matmul(
                    warm_ps[:], wjunk[:, :, 0:32], wjunk[:],
                    start=(wi == 0), stop=(wi == NWARM - 1),
                    perf_mode=DR,
                )

            # ---- resident weights / constants on the SP queue: tiny GN
            # operands first (needed ~t+13us), the big fp8 weights after so
            # they don't steal HBM bandwidth from the x0 stream.
            ones8 = wpool.tile([P, 2, 32], F8, name="ones8")
            nc.sync.dma_start(ones8[:], one8d[:])
            i1_s = wpool.tile([P, CT, GROUPS], F32, name="i1_s")
            nc.sync.dma_start(i1_s[:], i1d[:])
            i2_s = wpool.tile([GROUPS, C], F32, name="i2_s")
            nc.sync.dma_start(i2_s[:], i2d[:])
            gnw_s = wpool.tile([P, CT, 1], F32, name="gnw_s")
            nc.sync.dma_start(gnw_s[:], gnw[:])
            gnb_s = wpool.tile([P, CT, 1], F32, name="gnb_s")
            nc.sync.dma_start(gnb_s[:], gnb[:])
            eps_g = wpool.tile([GROUPS, 1], F32, name="eps_g")
            nc.gpsimd.memset(eps_g[:], EPS)
            ebias = wpool.tile([P, 1], F32, name="ebias")
            nc.gpsimd.memset(ebias[:], -ESHIFT)
            ones_k1 = wpool.tile([1, P], F32R, name="ones_k1")
            nc.sync.dma_start(ones_k1[:], one_r[:])
            wqk_s = wpool.tile([P, CT, C], F8, name="wqk_s")
            nc.sync.dma_start(wqk_s[:], wqk[:])
            wpv_s = wpool.tile([P, CT, C], F8, name="wpv_s")
            nc.sync.dma_start(wpv_s[:], wpv[:])
            if use_rowbias:
                g8_s = wpool.tile([P, CT, 1], F8, name="g8_s")
                nc.sync.dma_start(g8_s[:], g8d[:])
            if use_pb:
                pb8_s = wpool.tile([1, C], F8, name="pb8_s")
                nc.sync.dma_start(pb8_s[:], pb8d[:])

            def gn_chain(s, x_t):
                """GroupNorm in two phases of two chunks each: groups never
                span chunks and the indicator matmul zeroes unrelated group
                rows, so chunks 0/1 can be normalized while chunks 2/3 are
                still streaming in.  Vector: bn_stats + applies 2/3; Scalar:
                psum copies, ln/exp, applies 0/1; GpSimd: tiny tensor ops."""
                h_t = hpool.tile([P, CT, N], F8, tag="h", name=f"h_{s}")
                for ph, cis in enumerate(((0, 1), (2, 3))):
                    mv = aux.tile([P, 2, 2], F32, tag="mv", name=f"mv_{s}{ph}")
                    for k, ci in enumerate(cis):
                        st6 = aux.tile([P, 2, 6], F32, tag="st6",
                                       name=f"st6_{s}_{ci}")
                        nc.vector.bn_stats(st6[:, 0, :], x_t[:, ci, 0:512])
                        nc.vector.bn_stats(st6[:, 1, :], x_t[:, ci, 512:1024])
                        nc.vector.bn_aggr(mv[:, k, :], st6[:])
                    # mv col0 = mean, col1 := mean^2 + var = E[x^2]
                    msq = aux.tile([P, 2, 1], F32, tag="msq", name=f"msq_{s}{ph}")
                    nc.vector.tensor_tensor(
                        msq[:], mv[:, :, 0:1], mv[:, :, 0:1], OP.mult)
                    nc.vector.tensor_tensor(
                        mv[:, :, 1:2], mv[:, :, 1:2], msq[:], OP.add)
                    # group rows for these chunks; other rows land zero
                    psg = pdn.tile([GROUPS, 2], F32, tag="dn", name=f"psg_{s}{ph}")
                    for k, ci in enumerate(cis):
                        nc.tensor.matmul(
                            psg[:], i1_s[:, ci, :], mv[:, k, :],
                            start=(k == 0), stop=(k == 1))
                    # garr col0 = mean (Scalar copy), col1 = rstd (Exp out);
                    # Vector computes var straight from the psum
                    garr = aux.tile([GROUPS, 2], F32, tag="garr",
                                    name=f"garr_{s}{ph}")
                    nc.scalar.copy(garr[:, 0:1], psg[:, 0:1])
                    gsc = aux.tile([GROUPS, 2], F32, tag="gsc", name=f"gsc_{s}{ph}")
                    nc.scalar.activation(gsc[:, 0:1], psg[:, 0:1], AF.Square)
                    nc.vector.tensor_tensor(
                        gsc[:, 1:2], psg[:, 1:2], gsc[:, 0:1], OP.subtract)
                    # rstd = exp(-0.5 * ln(var + eps))
                    glv = aux.tile([GROUPS, 1], F32, tag="glv", name=f"glv_{s}{ph}")
                    nc.scalar.activation(glv[:], gsc[:, 1:2], AF.Ln, bias=eps_g[:])
                    nc.scalar.activation(garr[:, 1:2], glv[:], AF.Exp, scale=-0.5)
                    # broadcast back to these chunks' channels
                    psc = pdn.tile([P, 2, 2], F32, tag="dn", name=f"psc_{s}{ph}")
                    for k, ci in enumerate(cis):
                        nc.tensor.matmul(
                            psc[:, k, :], i2_s[:, ci * P:(ci + 1) * P],
                            garr[:, 0:2], start=True, stop=True)
                    scol = aux.tile([P, 2, 1], F32, tag="scol", name=f"scol_{s}{ph}")
                    bcol = aux.tile([P, 2, 1], F32, tag="bcol", name=f"bcol_{s}{ph}")
                    if triv_gn:
                        # gn_w == 1, gn_b == 0: scol = rstd, bcol = -mean*rstd
                        nc.vector.tensor_copy(scol[:], psc[:, :, 1:2])
                        msc = aux.tile([P, 2, 1], F32, tag="msc",
                                       name=f"msc_{s}{ph}")
                        nc.vector.tensor_tensor(
                            msc[:], psc[:, :, 0:1], scol[:], OP.mult)
                        nc.vector.tensor_scalar_mul(bcol[:], msc[:], -1.0)
                    else:
                        nc.vector.tensor_tensor(
                            scol[:], psc[:, :, 1:2],
                            gnw_s[:, 2 * ph:2 * ph + 2, :], OP.mult)
                        nc.vector.tensor_tensor(
                            bcol[:], psc[:, :, 0:1], scol[:], OP.mult)
                        nc.vector.tensor_tensor(
                            bcol[:], gnb_s[:, 2 * ph:2 * ph + 2, :], bcol[:],
                            OP.subtract)
                    for k, ci in enumerate(cis):
                        # first chunk of each phase on Scalar, second on
                        # Vector: the two applies run in parallel
                        if k == 0:
                            nc.scalar.activation(
                                h_t[:, ci, :], x_t[:, ci, :], AF.Identity,
                                bias=bcol[:, k, :], scale=scol[:, k, :])
                        else:
                            nc.vector.tensor_scalar(
                                h_t[:, ci, :], x_t[:, ci, :],
                                scol[:, k, :], bcol[:, k, :],
                                op0=OP.mult, op1=OP.add)
                return h_t

            def rowbias(s, h_t):
                """exp-bias per key (general path only; skipped when the
                host-folded g = Wk^T qb vector is exactly zero)."""
                ps_rb = pmm.tile([P, NT], F32, tag="mm", name=f"rbps_{s}")
                for mi in range(NT):
                    for kp in range(0, CT, 2):
                        nc.tensor.matmul(
                            ps_rb[:, mi:mi + 1],
                            h_t[:, kp:kp + 2, mi * P:(mi + 1) * P],
                            g8_s[:, kp:kp + 2, 0:1],
                            start=(kp == 0), stop=(kp == CT - 2),
                            perf_mode=DR,
                        )
                rowb = aux.tile([P, NT], F32, tag="rowb", name=f"rowb_{s}")
                nc.vector.tensor_scalar(
                    rowb[:], ps_rb[:], float(SCALE), float(-ESHIFT),
                    op0=OP.mult, op1=OP.add,
                )
                return rowb

            def t_mm(s, h_t):
                """t = M h (channel-major) matmuls; evacuation separate."""
                t_t = tpool.tile([P, CT, N], F8, tag="t", name=f"t_{s}")
                accs = []
                for mi in range(CT):
                    acc = pmm.tile([P, N], F32, tag="mm", name=f"tps_{s}_{mi}")
                    for kp in range(0, CT, 2):
                        for ni in range(NH):
                            nc.tensor.matmul(
                                acc[:, ni * 512:(ni + 1) * 512],
                                wqk_s[:, kp:kp + 2, mi * P:(mi + 1) * P],
                                h_t[:, kp:kp + 2, ni * 512:(ni + 1) * 512],
                                start=(kp == 0), stop=(kp == CT - 2),
                                perf_mode=DR,
                            )
                    accs.append(acc)
                return t_t, accs

            def t_evac(s, t_t, accs):
                for mi in range(CT):
                    nc.scalar.copy(t_t[:, mi, :], accs[mi][:])

            def v_mm(s, h_t, v_t=None, tps=range(0, NT, 2)):
                """v' = (Wp Wv) h, token-major; paired psum banks so each
                evacuation is one [P, 1024] DVE op.  Callable per tp-pair
                subset so other matmuls can interleave."""
                if v_t is None:
                    v_t = vpool.tile([P, NT, C], F8, tag="v", name=f"v_{s}")
                for tp in tps:
                    acc = pmm.tile([P, 2, 512], F32, tag="mm", name=f"vps_{s}_{tp}")
                    for sub in range(2):
                        for kp in range(0, CT, 2):
                            nc.tensor.matmul(
                                acc[:, sub, :],
                                h_t[:, kp:kp + 2, (tp + sub) * P:(tp + sub + 1) * P],
                                wpv_s[:, kp:kp + 2, :],
                                start=(kp == 0), stop=(kp == CT - 2),
                                perf_mode=DR,
                            )
                    nc.vector.tensor_copy(v_t[:, tp:tp + 2, :], acc[:])
                return v_t

            def scores(s, t_t, h_t, rowb):
                """S^T = t^T h; e = exp(SCALE*S + bias) in fp8 on Scalar."""
                e_t = epool.tile([P, NT, N], F8, tag="e", name=f"e_{s}")
                for mi in range(NT):
                    acc = pmm.tile([P, N], F32, tag="mm", name=f"sps_{s}_{mi}")
                    for kp in range(0, CT, 2):
                        for ni in range(NH):
                            nc.tensor.matmul(
                                acc[:, ni * 512:(ni + 1) * 512],
                                t_t[:, kp:kp + 2, mi * P:(mi + 1) * P],
                                h_t[:, kp:kp + 2, ni * 512:(ni + 1) * 512],
                                start=(kp == 0), stop=(kp == CT - 2),
                                perf_mode=DR,
                            )
                    bias = ebias[:] if rowb is None else rowb[:, mi:mi + 1]
                    nc.scalar.activation(
                        e_t[:, mi, :], acc[:], AF.Exp,
                        bias=bias, scale=float(SCALE),
                    )
                return e_t

            def dn_mm(s, e_t):
                """softmax denominators via all-ones DR matmul -> [32, N].
                ni-outer so the first half completes one group early and its
                reciprocal chain can start sooner."""
                dn = pdn.tile([32, N], F32, tag="dn", name=f"dn_{s}")
                for ni in range(NH):
                    for tp in range(0, NT, 2):
                        nc.tensor.matmul(
                            dn[:, ni * 512:(ni + 1) * 512],
                            ones8[:, 0:2, :],
                            e_t[:, tp:tp + 2, ni * 512:(ni + 1) * 512],
                            start=(tp == 0), stop=(tp == NT - 2),
                            perf_mode=DR,
                        )
                return dn

            def recip_ni(s, dn, ni):
                """1/dn for one 512-col half as exp(-ln(dn)) on Scalar."""
                lndn = rpool.tile([1, 512], F32, tag="lndn", name=f"lndn_{s}_{ni}")
                nc.scalar.activation(lndn[:], dn[0:1, ni * 512:(ni + 1) * 512], AF.Ln)
                recip = rpool.tile([1, 512], F32R, tag="recip", name=f"rec_{s}_{ni}")
                nc.scalar.activation(recip[:], lndn[:], AF.Exp, scale=-1.0)
                return recip

            def dn8_full(s, dn):
                dn8 = rpool.tile([1, N], F8, tag="dn8", name=f"dn8_{s}")
                nc.scalar.activation(
                    dn8[:], dn[0:1, :], AF.Copy, bias=0.0, scale=0.0625)
                return dn8

            def bcast_ni(s, rb_ps, recip, ni):
                nc.tensor.matmul(
                    rb_ps[:, ni * 512:(ni + 1) * 512], ones_k1[:],
                    recip[:], start=True, stop=True,
                )

            def rb_copy_ni(s, rb, rb_ps, ni):
                # Scalar, not Vector: the tail's mult cascade owns Vector
                nc.scalar.copy(
                    rb[:, ni * 512:(ni + 1) * 512],
                    rb_ps[:, ni * 512:(ni + 1) * 512])

            def av_group(s, v_t, e_t, mi, dn8):
                acc = pmm.tile([P, N], F32, tag="mm", name=f"avps_{s}_{mi}")
                last_pb = not use_pb
                for tp in range(0, NT, 2):
                    for ni in range(NH):
                        nc.tensor.matmul(
                            acc[:, ni * 512:(ni + 1) * 512],
                            v_t[:, tp:tp + 2, mi * P:(mi + 1) * P],
                            e_t[:, tp:tp + 2, ni * 512:(ni + 1) * 512],
                            start=(tp == 0),
                            stop=(last_pb and tp == NT - 2),
                            perf_mode=DR,
                        )
                if use_pb:
                    for ni in range(NH):
                        nc.tensor.matmul(
                            acc[:, ni * 512:(ni + 1) * 512],
                            pb8_s[:, mi * P:(mi + 1) * P],
                            dn8[:, ni * 512:(ni + 1) * 512],
                            start=False, stop=True,
                        )
                return acc

            def av_evac(s, acc, rb, mi, x_t, add_eng, dma_q, halves=False):
                """out = acc*rb + x for one channel block, streamed to HBM."""
                if not halves:
                    o1 = o1pool.tile([P, N], F32, tag="o1", name=f"o1_{s}_{mi}")
                    nc.vector.tensor_tensor(o1[:], acc[:], rb[:], OP.mult)
                    o_t = opool.tile([P, N], F32, tag="o", name=f"o_{s}_{mi}")
                    add_eng.tensor_tensor(o_t[:], o1[:], x_t[:, mi, :], OP.add)
                    dma_q.dma_start(out[s, mi * P:(mi + 1) * P, :], o_t[:])
                else:
                    adds = add_eng if isinstance(add_eng, list) else [add_eng] * NH
                    dmas = dma_q if isinstance(dma_q, list) else [dma_q] * NH
                    for ni in range(NH):
                        sl = slice(ni * 512, (ni + 1) * 512)
                        o1 = o1pool.tile([P, 512], F32, tag="o1",
                                         name=f"o1h_{s}_{mi}_{ni}")
                        nc.vector.tensor_tensor(o1[:], acc[:, sl], rb[:, sl], OP.mult)
                        o_t = opool.tile([P, 512], F32, tag="o",
                                         name=f"oh_{s}_{mi}_{ni}")
                        adds[ni].tensor_tensor(o_t[:], o1[:], x_t[:, mi, sl], OP.add)
                        dmas[ni].dma_start(
                            out[s, mi * P:(mi + 1) * P, sl], o_t[:])

            # ---- schedule ------------------------------------------------
            # Emission order == per-engine FIFO priority; arranged so no
            # queue holds a ready instruction behind a not-yet-ready one.
            ht0 = gn_chain(0, xt0)
            rowb0 = rowbias(0, ht0) if use_rowbias else None
            t0, t0acc = t_mm(0, ht0)
            t_evac(0, t0, t0acc)
            xt1 = load_x(1, [nc.sync])
            v0 = v_mm(0, ht0)
            ht1 = gn_chain(1, xt1)
            e0 = scores(0, t0, ht0, rowb0)
            rowb1 = rowbias(1, ht1) if use_rowbias else None
            t1, t1acc = t_mm(1, ht1)
            t_evac(1, t1, t1acc)
            # first half of v1 fills the PE while sample-0's exp drains;
            # dn0 slots in as soon as the last e0 tile lands
            v1 = v_mm(1, ht1, tps=(0, 2))
            dn0 = dn_mm(0, e0)
            dn80 = dn8_full(0, dn0) if use_pb else None
            rc00 = recip_ni(0, dn0, 0)
            rc01 = recip_ni(0, dn0, 1)
            v_mm(1, ht1, v_t=v1, tps=(4, 6))
            rb_ps0 = pdn.tile([P, N], F32, tag="dn", name="rbps_0")
            rb0 = rpool.tile([P, N], F32, tag="rb", name="rb_0")
            for ni, rc in ((0, rc00), (1, rc01)):
                bcast_ni(0, rb_ps0, rc, ni)
                rb_copy_ni(0, rb0, rb_ps0, ni)
            for mi in range(CT):
                acc = av_group(0, v0, e0, mi, dn80)
                av_evac(0, acc, rb0, mi, xt0, nc.gpsimd, nc.sync)
            e1 = scores(1, t1, ht1, rowb1)
            # interleave sample-1 AV groups around the dn/recip chain so the
            # PE stays busy while the softmax denominators come out
            if use_pb:
                dn1 = dn_mm(1, e1)
                dn81 = dn8_full(1, dn1)
                acc10 = av_group(1, v1, e1, 0, dn81)
                acc11 = av_group(1, v1, e1, 1, dn81)
            else:
                dn81 = None
                acc10 = av_group(1, v1, e1, 0, None)
                acc11 = av_group(1, v1, e1, 1, None)
                dn1 = dn_mm(1, e1)
            rb_ps1 = pdn.tile([P, N], F32, tag="dn", name="rbps_1")
            rb1 = rpool.tile([P, N], F32, tag="rb", name="rb_1")
            rc10 = recip_ni(1, dn1, 0)
            acc12 = av_group(1, v1, e1, 2, dn81)
            bcast_ni(1, rb_ps1, rc10, 0)
            rb_copy_ni(1, rb1, rb_ps1, 0)
            rc11 = recip_ni(1, dn1, 1)
            acc13 = av_group(1, v1, e1, 3, dn81)
            bcast_ni(1, rb_ps1, rc11, 1)
            rb_copy_ni(1, rb1, rb_ps1, 1)
            av_evac(1, acc10, rb1, 0, xt1, nc.gpsimd, nc.sync, halves=True)
            av_evac(1, acc11, rb1, 1, xt1, nc.gpsimd, nc.sync, halves=True)
            av_evac(1, acc12, rb1, 2, xt1, nc.vector, nc.sync, halves=True)
            av_evac(1, acc13, rb1, 3, xt1, [nc.gpsimd, nc.vector],
                    [nc.sync, nc.scalar], halves=True)

        ctx_lp.__exit__(None, None, None)
    if split_waits:
        _split_multi_waits(nc)
    return nc


_CACHE_LOCK = threading.Lock()
_NC_CACHE = {}


def _get_program(use_rowbias, use_pb, triv_gn):
    key = (use_rowbias, use_pb, triv_gn)
    with _CACHE_LOCK:
        if key not in _NC_CACHE:
            _NC_CACHE[key] = _build_program(
                use_rowbias=use_rowbias, use_pb=use_pb, triv_gn=triv_gn)
        return _NC_CACHE[key]


def _prep_weights(gn_w, gn_b, qkv_w, qkv_b, proj_w, proj_b):
    def pt(v):  # [C] -> [P, CT] with c = t*P + p
        return np.ascontiguousarray(v.reshape(CT, P).T)

    def wt(m):  # [C_out, C_in] -> lhsT layout [P, CT, C_out]
        return np.ascontiguousarray(m.T.reshape(CT, P, m.shape[0]).transpose(1, 0, 2))

    Wq = qkv_w[0:C]
    Wk = qkv_w[C:2 * C]
    Wv = qkv_w[2 * C:3 * C]
    M = Wq.T @ Wk
    Wpv = proj_w @ Wv
    g = Wk.T @ qkv_b[0:C]
    pb_eff = proj_b + proj_w @ qkv_b[2 * C:3 * C]

    use_rowbias = bool(np.abs(g).max() > 0)
    use_pb = bool(np.abs(pb_eff).max() > 0)
    triv_gn = bool(np.all(gn_w == 1.0) and np.all(gn_b == 0.0))

    ind1 = np.zeros((C, GROUPS), np.float32)
    ind1[np.arange(C), np.arange(C) // GSIZE] = 1.0 / GSIZE
    ind2 = np.zeros((GROUPS, C), np.float32)
    ind2[np.arange(C) // GSIZE, np.arange(C)] = 1.0

    w = {
        "wqk": wt(M).astype(NPF8),
        "wpv": wt(Wpv).astype(NPF8),
        "one8": np.ones((P, 2, 32), dtype=NPF8),
        "one_r": np.ones((1, P), np.float32),
        "gnw": pt(gn_w)[:, :, None],
        "gnb": pt(gn_b)[:, :, None],
        "ind1": np.ascontiguousarray(
            ind1.reshape(CT, P, GROUPS).transpose(1, 0, 2)),
        "ind2": ind2,
    }
    if use_rowbias:
        w["g8"] = pt(g)[:, :, None].astype(NPF8)
    if use_pb:
        w["pb8"] = (16.0 * pb_eff)[None, :].astype(NPF8)
    return w, use_rowbias, use_pb, triv_gn


def kernel(x, gn_w, gn_b, qkv_w, qkv_b, proj_w, proj_b):
    x = np.asarray(x, dtype=np.float32)
    weights, use_rowbias, use_pb, triv_gn = _prep_weights(
        np.asarray(gn_w, np.float32), np.asarray(gn_b, np.float32),
        np.asarray(qkv_w, np.float32), np.asarray(qkv_b, np.float32),
        np.asarray(proj_w, np.float32), np.asarray(proj_b, np.float32),
    )

    xr = x.reshape(B, C, N).astype(NPBF16)
    in_maps = []
    for core in range(NCORES):
        m = dict(weights)
        m["xs"] = np.ascontiguousarray(xr[core * SPC:(core + 1) * SPC])
        in_maps.append(m)

    nc = _get_program(use_rowbias, use_pb, triv_gn)
    trace = bool(int(os.environ.get("BASS_KERNEL_TRACE", "0")))
    kwargs = {}
    if trace:
        kwargs["trace"] = True
        kwargs["tmpdir"] = os.environ.get("BASS_KERNEL_TRACE_DIR") or None
    res = run_bass_kernel_spmd(nc, in_maps, core_ids=list(range(NCORES)), **kwargs)
    if trace:
        kernel.last_results = res

    out = np.concatenate([res.results[i]["out"] for i in range(NCORES)], axis=0)
    return out.reshape(B, C, H, W)
